# revision 18
# baseline (speedup 1.0000x reference)
"""Trainium2 Bass kernel for nn_AttentionModuleBiModal (B=4, N1=N2=8192).

Math (per batch b):
    tm2[j] = w0*m2[j] + b0
    M[i,j] = tanh(m1[i] * tm2[j])                      (never materialized in HBM)
    s1[i]  = sum_j (w2*m2[j] + b2) * M[i,j]
    s2[j]  = sum_i (w1*m1[i] + b1) * M[i,j]
    a_m1 = tanh(w1*m1 + b1 + s1);  a_m2 = tanh(w2*m2 + b2 + s2)
    out1 = softmax(a_m1*w3 + b3) * m1;  out2 = softmax(a_m2*w4 + b4) * m2

Sharding: 8 cores = 4 batches x 2 halves of the m2 (j) dimension. Each core
computes, for its 4096 j's and all 8192 i's: the partial s1 (summed over its
j-half) and the complete s2 for its j-half. Layout on device: j on SBUF
partitions, i along the free dim (2 passes of 4096).

Per 128-j tile the scalar engine emits T = tanh(tm2[j] * m1[i]) in fp16; the
tensor engine contracts T against wm2[j] into PSUM (s1); the vector engine's
fused scalar_tensor_tensor computes sum_i wm1[i]*T[j,i] (s2) via its
accumulator. The O(N) epilogue (tanh, softmax, scaling) runs on host in
float64 - it is 0.01% of the FLOPs.
"""

import numpy as np

B = 4
N = 8192
NCORES = 8
JHALF = N // 2          # j-range per core
NJT = JHALF // 128      # 32 j-tiles
IPASS = 2
ICH = N // IPASS        # 4096 i per pass
MMN = 512               # matmul moving free-dim chunk (one PSUM bank)
WARMUP_MM = 16          # dense matmul burst to flip the PE HAM gate to 8/8
# Tiles whose weighted reduction is offloaded from the DVE (1x fused-accum
# scalar_tensor_tensor) to ACT (Copy with accumulator) fed by a DVE 2x
# tensor_tensor product. Balances the two engines' serial chains.
OFFLOAD_EVERY = 7       # (p*NJT+jt) % OFFLOAD_EVERY == 3 -> ACT path (~9 tiles)

_CACHE = {}

_SCALARS = ("w0", "b0", "w1", "b1", "w2", "b2", "w3", "b3", "w4", "b4")


def _build_program():
    from contextlib import ExitStack

    import concourse.bacc as bacc
    import concourse.tile as tile
    from concourse import mybir

    f32, f16 = mybir.dt.float32, mybir.dt.float16
    nc = bacc.Bacc("TRN2", target_bir_lowering=False, debug=False)

    d_m1h = nc.dram_tensor("m1h", [N], f16, kind="ExternalInput")
    d_wm1 = nc.dram_tensor("wm1", [N], f16, kind="ExternalInput")
    d_tm2 = nc.dram_tensor("tm2", [JHALF], f32, kind="ExternalInput")
    d_wm2 = nc.dram_tensor("wm2", [JHALF], f16, kind="ExternalInput")

    d_s1 = nc.dram_tensor("o_s1", [N], f32, kind="ExternalOutput")
    # +1 aux column: the first tile of pass 0 is split in two halves so the
    # first tanh only waits on a quarter of the broadcast DMA; its second
    # half accumulates into the aux column (host adds it back into col 0).
    d_s2 = nc.dram_tensor("o_s2", [128, IPASS * NJT + 1], f32, kind="ExternalOutput")

    with ExitStack() as ctx:
        tc = ctx.enter_context(tile.TileContext(nc))
        singles = ctx.enter_context(tc.tile_pool(name="singles", bufs=1))
        bcp = ctx.enter_context(tc.tile_pool(name="bcp", bufs=2))
        tp = ctx.enter_context(tc.tile_pool(name="tp", bufs=4))
        sp = ctx.enter_context(tc.tile_pool(name="sp", bufs=2))
        pp = ctx.enter_context(tc.tile_pool(name="pp", bufs=1, space="PSUM"))

        # per-partition scale/weight vectors: sbuf[p, jt] = v[jt*128 + p]
        tm2_sb = singles.tile([128, NJT], f32)
        nc.sync.dma_start(
            out=tm2_sb, in_=d_tm2.ap().rearrange("(jt p) -> p jt", p=128)
        )
        wm2_sb = singles.tile([128, NJT], f16)
        nc.sync.dma_start(
            out=wm2_sb, in_=d_wm2.ap().rearrange("(jt p) -> p jt", p=128)
        )
        accS2 = singles.tile([128, IPASS * NJT + 1], f32)
        # aux col is only written when the split-tile path is active; the
        # final DMA reads all columns, so zero it.
        nc.vector.memset(accS2[:, IPASS * NJT :], 0)

        # PE warm-up fodder (zeros; results overwritten by the real matmuls)
        wrm_w = singles.tile([128, 1], f16)
        nc.vector.memset(wrm_w, 0)
        wrm_x = singles.tile([128, MMN], f16)
        nc.vector.memset(wrm_x, 0)

        for p in range(IPASS):
            lo = p * ICH
            mbh = bcp.tile([128, ICH], f16, tag="mbh")
            nc.sync.dma_start(
                out=mbh, in_=d_m1h.ap()[lo : lo + ICH].partition_broadcast(128)
            )
            wb16 = bcp.tile([128, ICH], f16, tag="wb16")
            nc.sync.dma_start(
                out=wb16, in_=d_wm1.ap()[lo : lo + ICH].partition_broadcast(128)
            )

            # [1, ICH] fp32 pads to [128, ICH] = all 8 PSUM banks; each MMN
            # slice is bank-aligned so each matmul accumulates within a bank.
            ps1 = pp.tile([1, ICH], f32, tag="s1", name="ps1")

            if p == 0:
                # Dense burst so the HAM activity monitor lifts the PE clock
                # gate to 8/8 before the steady-state matmul cadence begins.
                for _ in range(WARMUP_MM):
                    nc.tensor.matmul(
                        ps1[0:1, 0:MMN], lhsT=wrm_w, rhs=wrm_x,
                        start=True, stop=True,
                    )

            pending_copy = []
            for jt in range(NJT):
                col = p * NJT + jt
                segments = [(0, ICH, col)]
                for off, w, segcol in segments:
                    T = tp.tile([128, w], f16, tag="T", name="T")
                    nc.scalar.activation(
                        out=T,
                        in_=mbh[:, off : off + w],
                        func=mybir.ActivationFunctionType.Tanh,
                        scale=tm2_sb[:, jt : jt + 1],
                    )
                    # Emit deferred ACT-side reductions AFTER this tanh so a
                    # stalled Copy never blocks tanh issue in ACT's FIFO.
                    while pending_copy and pending_copy[0][0] <= jt - 2:
                        _, W, wcol = pending_copy.pop(0)
                        scrC = sp.tile([128, ICH], f16, tag="scrC", name="scrC")
                        nc.scalar.activation(
                            out=scrC,
                            in_=W,
                            func=mybir.ActivationFunctionType.Copy,
                            accum_out=accS2[:, wcol : wcol + 1],
                        )
                    for k in range(w // MMN):
                        nc.tensor.matmul(
                            ps1[0:1, off + k * MMN : off + (k + 1) * MMN],
                            lhsT=wm2_sb[:, jt : jt + 1],
                            rhs=T[:, k * MMN : (k + 1) * MMN],
                            start=(jt == 0),
                            stop=(jt == NJT - 1),
                        )
                    if col % OFFLOAD_EVERY == 3 and len(segments) == 1:
                        W = sp.tile([128, ICH], f16, tag="scrW", name="scrW")
                        nc.vector.tensor_tensor(
                            out=W, in0=T, in1=wb16, op=mybir.AluOpType.mult
                        )
                        pending_copy.append((jt, W, col))
                    else:
                        scr = sp.tile([128, w], f16, tag="scr", name="scr")
                        nc.vector.scalar_tensor_tensor(
                            out=scr,
                            in0=T,
                            scalar=1.0,
                            in1=wb16[:, off : off + w],
                            op0=mybir.AluOpType.mult,
                            op1=mybir.AluOpType.mult,
                            accum_out=accS2[:, segcol : segcol + 1],
                        )
            for _, W, wcol in pending_copy:
                scrC = sp.tile([128, ICH], f16, tag="scrC", name="scrC")
                nc.scalar.activation(
                    out=scrC,
                    in_=W,
                    func=mybir.ActivationFunctionType.Copy,
                    accum_out=accS2[:, wcol : wcol + 1],
                )

            s1sb = sp.tile([1, ICH], f32, tag="s1sb", name="s1sb")
            if p == 0:
                nc.scalar.copy(out=s1sb[0:1, :], in_=ps1[0:1, :])
            else:
                # DVE is idle at the tail; keep the final evac off ACT
                nc.vector.tensor_copy(out=s1sb[0:1, :], in_=ps1[0:1, :])
            nc.sync.dma_start(
                out=d_s1.ap()[lo : lo + ICH].unsqueeze(0), in_=s1sb[0:1, :]
            )

        nc.sync.dma_start(out=d_s2.ap(), in_=accS2[:, :])

    nc.compile()
    return nc


def _get_program():
    if "nc" not in _CACHE:
        _CACHE["nc"] = _build_program()
    return _CACHE["nc"]


def _make_in_maps(m1, m2, sc):
    in_maps = []
    for c in range(NCORES):
        b, h = divmod(c, 2)
        js = slice(h * JHALF, (h + 1) * JHALF)
        m2s = m2[b, js].astype(np.float64)
        m1s = m1[b].astype(np.float64)
        in_maps.append(
            {
                "m1h": m1[b].astype(np.float16),
                "wm1": (sc["w1"] * m1s + sc["b1"]).astype(np.float16),
                "tm2": (sc["w0"] * m2s + sc["b0"]).astype(np.float32),
                "wm2": (sc["w2"] * m2s + sc["b2"]).astype(np.float16),
            }
        )
    return in_maps


def _run_device(inputs, trace=False):
    from concourse.bass_utils import run_bass_kernel_spmd

    nc = _get_program()
    m1 = np.asarray(inputs["m1_t"], np.float32)[..., 0]  # [B, N]
    m2 = np.asarray(inputs["m2_t"], np.float32)[..., 0]
    sc = {k: float(np.asarray(inputs[k])) for k in _SCALARS}
    in_maps = _make_in_maps(m1, m2, sc)
    res = run_bass_kernel_spmd(nc, in_maps, list(range(NCORES)), trace=trace)
    return res, m1, m2, sc


def _postprocess(results, m1, m2, sc):
    out1 = np.zeros((B, N), np.float32)
    out2 = np.zeros((B, N), np.float32)
    for b in range(B):
        halves = [results[2 * b], results[2 * b + 1]]
        s1 = (
            halves[0]["o_s1"].astype(np.float64)
            + halves[1]["o_s1"].astype(np.float64)
        )

        def s2_of(r):
            S = r["o_s2"].astype(np.float64)
            S[:, 0] += S[:, IPASS * NJT]  # aux col: 2nd half of split tile 0
            S = S[:, :NJT] + S[:, NJT : IPASS * NJT]  # sum passes; j = jt*128+p
            return S.T.reshape(-1)        # [JHALF]

        s2 = np.concatenate([s2_of(halves[0]), s2_of(halves[1])])

        m1b = m1[b].astype(np.float64)
        m2b = m2[b].astype(np.float64)
        a_m1 = np.tanh(sc["w1"] * m1b + sc["b1"] + s1)
        a_m2 = np.tanh(sc["w2"] * m2b + sc["b2"] + s2)
        l1 = a_m1 * sc["w3"] + sc["b3"]
        l2 = a_m2 * sc["w4"] + sc["b4"]
        e1 = np.exp(l1 - l1.max())
        e2 = np.exp(l2 - l2.max())
        out1[b] = (e1 / e1.sum() * m1b).astype(np.float32)
        out2[b] = (e2 / e2.sum() * m2b).astype(np.float32)
    return out1, out2


def kernel(**inputs):
    res, m1, m2, sc = _run_device(inputs, trace=False)
    return _postprocess(res.results, m1, m2, sc)


# revision 20
# speedup vs baseline: 1.0025x; 1.0025x over previous
"""Trainium2 Bass kernel for nn_AttentionModuleBiModal (B=4, N1=N2=8192).

Math (per batch b):
    tm2[j] = w0*m2[j] + b0
    M[i,j] = tanh(m1[i] * tm2[j])                      (never materialized in HBM)
    s1[i]  = sum_j (w2*m2[j] + b2) * M[i,j]
    s2[j]  = sum_i (w1*m1[i] + b1) * M[i,j]
    a_m1 = tanh(w1*m1 + b1 + s1);  a_m2 = tanh(w2*m2 + b2 + s2)
    out1 = softmax(a_m1*w3 + b3) * m1;  out2 = softmax(a_m2*w4 + b4) * m2

Sharding: 8 cores = 4 batches x 2 halves of the m2 (j) dimension. Each core
computes, for its 4096 j's and all 8192 i's: the partial s1 (summed over its
j-half) and the complete s2 for its j-half. Layout on device: j on SBUF
partitions, i along the free dim (2 passes of 4096).

Per 128-j tile the scalar engine emits T = tanh(tm2[j] * m1[i]) in fp16; the
tensor engine contracts T against wm2[j] into PSUM (s1); the vector engine's
fused scalar_tensor_tensor computes sum_i wm1[i]*T[j,i] (s2) via its
accumulator. The O(N) epilogue (tanh, softmax, scaling) runs on host in
float64 - it is 0.01% of the FLOPs.
"""

import numpy as np

B = 4
N = 8192
NCORES = 8
JHALF = N // 2          # j-range per core
NJT = JHALF // 128      # 32 j-tiles
IPASS = 2
ICH = N // IPASS        # 4096 i per pass
MMN = 512               # matmul moving free-dim chunk (one PSUM bank)
WARMUP_MM = 16          # dense matmul burst to flip the PE HAM gate to 8/8
# Tiles whose weighted reduction is offloaded from the DVE (1x fused-accum
# scalar_tensor_tensor) to ACT (Copy with accumulator) fed by a DVE 2x
# tensor_tensor product. Balances the two engines' serial chains.
OFFLOAD_EVERY = 7       # (p*NJT+jt) % OFFLOAD_EVERY == 3 -> ACT path (~9 tiles)

_CACHE = {}

_SCALARS = ("w0", "b0", "w1", "b1", "w2", "b2", "w3", "b3", "w4", "b4")


def _build_program():
    from contextlib import ExitStack

    import concourse.bacc as bacc
    import concourse.tile as tile
    from concourse import mybir

    f32, f16 = mybir.dt.float32, mybir.dt.float16
    nc = bacc.Bacc("TRN2", target_bir_lowering=False, debug=False)

    d_m1h = nc.dram_tensor("m1h", [N], f16, kind="ExternalInput")
    d_wm1 = nc.dram_tensor("wm1", [N], f16, kind="ExternalInput")
    d_tm2 = nc.dram_tensor("tm2", [JHALF], f32, kind="ExternalInput")
    d_wm2 = nc.dram_tensor("wm2", [JHALF], f16, kind="ExternalInput")

    d_s1 = nc.dram_tensor("o_s1", [N], f32, kind="ExternalOutput")
    # +1 aux column: the first tile of pass 0 is split in two halves so the
    # first tanh only waits on a quarter of the broadcast DMA; its second
    # half accumulates into the aux column (host adds it back into col 0).
    d_s2 = nc.dram_tensor("o_s2", [128, IPASS * NJT + 1], f32, kind="ExternalOutput")

    with ExitStack() as ctx:
        tc = ctx.enter_context(tile.TileContext(nc))
        singles = ctx.enter_context(tc.tile_pool(name="singles", bufs=1))
        bcp = ctx.enter_context(tc.tile_pool(name="bcp", bufs=2))
        tp = ctx.enter_context(tc.tile_pool(name="tp", bufs=4))
        sp = ctx.enter_context(tc.tile_pool(name="sp", bufs=2))
        pp = ctx.enter_context(tc.tile_pool(name="pp", bufs=1, space="PSUM"))

        # per-partition scale/weight vectors: sbuf[p, jt] = v[jt*128 + p]
        tm2_sb = singles.tile([128, NJT], f32)
        nc.sync.dma_start(
            out=tm2_sb, in_=d_tm2.ap().rearrange("(jt p) -> p jt", p=128)
        )
        wm2_sb = singles.tile([128, NJT], f16)
        nc.sync.dma_start(
            out=wm2_sb, in_=d_wm2.ap().rearrange("(jt p) -> p jt", p=128)
        )
        accS2 = singles.tile([128, IPASS * NJT + 1], f32)
        # aux col is only written when the split-tile path is active; the
        # final DMA reads all columns, so zero it.
        nc.vector.memset(accS2[:, IPASS * NJT :], 0)

        # PE warm-up fodder (zeros; results overwritten by the real matmuls)
        wrm_w = singles.tile([128, 1], f16)
        nc.vector.memset(wrm_w, 0)
        wrm_x = singles.tile([128, MMN], f16)
        nc.vector.memset(wrm_x, 0)

        for p in range(IPASS):
            lo = p * ICH
            mbh = bcp.tile([128, ICH], f16, tag="mbh")
            nc.sync.dma_start(
                out=mbh, in_=d_m1h.ap()[lo : lo + ICH].partition_broadcast(128)
            )
            wb16 = bcp.tile([128, ICH], f16, tag="wb16")
            nc.sync.dma_start(
                out=wb16, in_=d_wm1.ap()[lo : lo + ICH].partition_broadcast(128)
            )

            # [1, ICH] fp32 pads to [128, ICH] = all 8 PSUM banks; each MMN
            # slice is bank-aligned so each matmul accumulates within a bank.
            ps1 = pp.tile([1, ICH], f32, tag="s1", name="ps1")

            if p == 0:
                # Dense burst so the HAM activity monitor lifts the PE clock
                # gate to 8/8 before the steady-state matmul cadence begins.
                for _ in range(WARMUP_MM):
                    nc.tensor.matmul(
                        ps1[0:1, 0:MMN], lhsT=wrm_w, rhs=wrm_x,
                        start=True, stop=True,
                    )

            pending_copy = []
            for jt in range(NJT):
                col = p * NJT + jt
                segments = [(0, ICH, col)]
                for off, w, segcol in segments:
                    T = tp.tile([128, w], f16, tag="T", name="T")
                    nc.scalar.activation(
                        out=T,
                        in_=mbh[:, off : off + w],
                        func=mybir.ActivationFunctionType.Tanh,
                        scale=tm2_sb[:, jt : jt + 1],
                    )
                    # Emit deferred ACT-side reductions AFTER this tanh so a
                    # stalled Copy never blocks tanh issue in ACT's FIFO.
                    while pending_copy and pending_copy[0][0] <= jt - 2:
                        _, W, wcol = pending_copy.pop(0)
                        scrC = sp.tile([128, ICH], f16, tag="scrC", name="scrC")
                        nc.scalar.activation(
                            out=scrC,
                            in_=W,
                            func=mybir.ActivationFunctionType.Copy,
                            accum_out=accS2[:, wcol : wcol + 1],
                        )
                    for k in range(w // MMN):
                        nc.tensor.matmul(
                            ps1[0:1, off + k * MMN : off + (k + 1) * MMN],
                            lhsT=wm2_sb[:, jt : jt + 1],
                            rhs=T[:, k * MMN : (k + 1) * MMN],
                            start=(jt == 0),
                            stop=(jt == NJT - 1),
                        )
                    if col % OFFLOAD_EVERY == 3 and len(segments) == 1:
                        W = sp.tile([128, ICH], f16, tag="scrW", name="scrW")
                        nc.vector.tensor_tensor(
                            out=W, in0=T, in1=wb16, op=mybir.AluOpType.mult
                        )
                        pending_copy.append((jt, W, col))
                    else:
                        scr = sp.tile([128, w], f16, tag="scr", name="scr")
                        nc.vector.scalar_tensor_tensor(
                            out=scr,
                            in0=T,
                            scalar=1.0,
                            in1=wb16[:, off : off + w],
                            op0=mybir.AluOpType.mult,
                            op1=mybir.AluOpType.mult,
                            accum_out=accS2[:, segcol : segcol + 1],
                        )
            for _, W, wcol in pending_copy:
                scrC = sp.tile([128, ICH], f16, tag="scrC", name="scrC")
                nc.scalar.activation(
                    out=scrC,
                    in_=W,
                    func=mybir.ActivationFunctionType.Copy,
                    accum_out=accS2[:, wcol : wcol + 1],
                )

            s1sb = sp.tile([1, ICH], f32, tag="s1sb", name="s1sb")
            if p == 0:
                nc.scalar.copy(out=s1sb[0:1, :], in_=ps1[0:1, :])
            else:
                # DVE is idle at the tail; keep the final evac off ACT
                nc.vector.tensor_copy(out=s1sb[0:1, :], in_=ps1[0:1, :])
            nc.sync.dma_start(
                out=d_s1.ap()[lo : lo + ICH].unsqueeze(0), in_=s1sb[0:1, :]
            )

        nc.sync.dma_start(out=d_s2.ap(), in_=accS2[:, :])

    nc.compile()
    return nc


def _get_program():
    if "nc" not in _CACHE:
        _CACHE["nc"] = _build_program()
    return _CACHE["nc"]


def _make_in_maps(m1, m2, sc):
    in_maps = []
    for c in range(NCORES):
        b, h = divmod(c, 2)
        js = slice(h * JHALF, (h + 1) * JHALF)
        m2s = m2[b, js].astype(np.float64)
        m1s = m1[b].astype(np.float64)
        in_maps.append(
            {
                "m1h": m1[b].astype(np.float16),
                "wm1": (sc["w1"] * m1s + sc["b1"]).astype(np.float16),
                "tm2": (sc["w0"] * m2s + sc["b0"]).astype(np.float32),
                "wm2": (sc["w2"] * m2s + sc["b2"]).astype(np.float16),
            }
        )
    return in_maps


def _run_device(inputs, trace=False):
    from concourse.bass_utils import run_bass_kernel_spmd

    nc = _get_program()
    m1 = np.asarray(inputs["m1_t"], np.float32)[..., 0]  # [B, N]
    m2 = np.asarray(inputs["m2_t"], np.float32)[..., 0]
    sc = {k: float(np.asarray(inputs[k])) for k in _SCALARS}
    in_maps = _make_in_maps(m1, m2, sc)
    res = run_bass_kernel_spmd(nc, in_maps, list(range(NCORES)), trace=trace)
    return res, m1, m2, sc


def _postprocess(results, m1, m2, sc):
    out1 = np.zeros((B, N), np.float32)
    out2 = np.zeros((B, N), np.float32)
    for b in range(B):
        halves = [results[2 * b], results[2 * b + 1]]
        s1 = (
            halves[0]["o_s1"].astype(np.float64)
            + halves[1]["o_s1"].astype(np.float64)
        )

        def s2_of(r):
            S = r["o_s2"].astype(np.float64)
            S[:, 0] += S[:, IPASS * NJT]  # aux col: 2nd half of split tile 0
            S = S[:, :NJT] + S[:, NJT : IPASS * NJT]  # sum passes; j = jt*128+p
            return S.T.reshape(-1)        # [JHALF]

        s2 = np.concatenate([s2_of(halves[0]), s2_of(halves[1])])

        m1b = m1[b].astype(np.float64)
        m2b = m2[b].astype(np.float64)
        a_m1 = np.tanh(sc["w1"] * m1b + sc["b1"] + s1)
        a_m2 = np.tanh(sc["w2"] * m2b + sc["b2"] + s2)
        l1 = a_m1 * sc["w3"] + sc["b3"]
        l2 = a_m2 * sc["w4"] + sc["b4"]
        e1 = np.exp(l1 - l1.max())
        e2 = np.exp(l2 - l2.max())
        out1[b] = (e1 / e1.sum() * m1b).astype(np.float32)
        out2[b] = (e2 / e2.sum() * m2b).astype(np.float32)
    return out1, out2


def kernel(**inputs):
    res, m1, m2, sc = _run_device(inputs, trace=False)
    return _postprocess(res.results, m1, m2, sc)


# revision 22
# speedup vs baseline: 1.0036x; 1.0011x over previous
"""Trainium2 Bass kernel for nn_AttentionModuleBiModal (B=4, N1=N2=8192).

Math (per batch b):
    tm2[j] = w0*m2[j] + b0
    M[i,j] = tanh(m1[i] * tm2[j])                      (never materialized in HBM)
    s1[i]  = sum_j (w2*m2[j] + b2) * M[i,j]
    s2[j]  = sum_i (w1*m1[i] + b1) * M[i,j]
    a_m1 = tanh(w1*m1 + b1 + s1);  a_m2 = tanh(w2*m2 + b2 + s2)
    out1 = softmax(a_m1*w3 + b3) * m1;  out2 = softmax(a_m2*w4 + b4) * m2

Sharding: 8 cores = 4 batches x 2 halves of the m2 (j) dimension. Each core
computes, for its 4096 j's and all 8192 i's: the partial s1 (summed over its
j-half) and the complete s2 for its j-half. Layout on device: j on SBUF
partitions, i along the free dim (2 passes of 4096).

Per 128-j tile the scalar engine emits T = tanh(tm2[j] * m1[i]) in fp16; the
tensor engine contracts T against wm2[j] into PSUM (s1); the vector engine's
fused scalar_tensor_tensor computes sum_i wm1[i]*T[j,i] (s2) via its
accumulator. The O(N) epilogue (tanh, softmax, scaling) runs on host in
float64 - it is 0.01% of the FLOPs.
"""

import numpy as np

B = 4
N = 8192
NCORES = 8
JHALF = N // 2          # j-range per core
NJT = JHALF // 128      # 32 j-tiles
IPASS = 2
ICH = N // IPASS        # 4096 i per pass
MMN = 512               # matmul moving free-dim chunk (one PSUM bank)
WARMUP_MM = 16          # dense matmul burst to flip the PE HAM gate to 8/8
# Tiles whose weighted reduction is offloaded from the DVE (1x fused-accum
# scalar_tensor_tensor) to ACT (Copy with accumulator) fed by a DVE 2x
# tensor_tensor product. Balances the two engines' serial chains.
OFFLOAD_EVERY = 7       # (p*NJT+jt) % OFFLOAD_EVERY == 3 -> ACT path (~9 tiles)

_CACHE = {}

_SCALARS = ("w0", "b0", "w1", "b1", "w2", "b2", "w3", "b3", "w4", "b4")


def _build_program():
    from contextlib import ExitStack

    import concourse.bacc as bacc
    import concourse.tile as tile
    from concourse import mybir

    f32, f16 = mybir.dt.float32, mybir.dt.float16
    nc = bacc.Bacc("TRN2", target_bir_lowering=False, debug=False)

    d_m1h = nc.dram_tensor("m1h", [N], f16, kind="ExternalInput")
    d_wm1 = nc.dram_tensor("wm1", [N], f16, kind="ExternalInput")
    d_tm2 = nc.dram_tensor("tm2", [JHALF], f32, kind="ExternalInput")
    d_wm2 = nc.dram_tensor("wm2", [JHALF], f16, kind="ExternalInput")

    d_s1 = nc.dram_tensor("o_s1", [N], f32, kind="ExternalOutput")
    # +1 aux column: the first tile of pass 0 is split in two halves so the
    # first tanh only waits on a quarter of the broadcast DMA; its second
    # half accumulates into the aux column (host adds it back into col 0).
    d_s2 = nc.dram_tensor("o_s2", [128, IPASS * NJT + 1], f32, kind="ExternalOutput")

    with ExitStack() as ctx:
        tc = ctx.enter_context(tile.TileContext(nc))
        singles = ctx.enter_context(tc.tile_pool(name="singles", bufs=1))
        bcp = ctx.enter_context(tc.tile_pool(name="bcp", bufs=2))
        tp = ctx.enter_context(tc.tile_pool(name="tp", bufs=4))
        sp = ctx.enter_context(tc.tile_pool(name="sp", bufs=2))
        pp = ctx.enter_context(tc.tile_pool(name="pp", bufs=1, space="PSUM"))

        # per-partition scale/weight vectors: sbuf[p, jt] = v[jt*128 + p]
        tm2_sb = singles.tile([128, NJT], f32)
        nc.sync.dma_start(
            out=tm2_sb, in_=d_tm2.ap().rearrange("(jt p) -> p jt", p=128)
        )
        wm2_sb = singles.tile([128, NJT], f16)
        nc.sync.dma_start(
            out=wm2_sb, in_=d_wm2.ap().rearrange("(jt p) -> p jt", p=128)
        )
        accS2 = singles.tile([128, IPASS * NJT + 1], f32)
        # aux col is only written when the split-tile path is active; the
        # final DMA reads all columns, so zero it.
        nc.vector.memset(accS2[:, IPASS * NJT :], 0)

        # PE warm-up fodder (zeros; results overwritten by the real matmuls)
        wrm_w = singles.tile([128, 1], f16)
        nc.vector.memset(wrm_w, 0)
        wrm_x = singles.tile([128, MMN], f16)
        nc.vector.memset(wrm_x, 0)

        for p in range(IPASS):
            lo = p * ICH
            mbh = bcp.tile([128, ICH], f16, tag="mbh")
            nc.sync.dma_start(
                out=mbh, in_=d_m1h.ap()[lo : lo + ICH].partition_broadcast(128)
            )
            wb16 = bcp.tile([128, ICH], f16, tag="wb16")
            nc.sync.dma_start(
                out=wb16, in_=d_wm1.ap()[lo : lo + ICH].partition_broadcast(128)
            )

            # [1, ICH] fp32 pads to [128, ICH] = all 8 PSUM banks; each MMN
            # slice is bank-aligned so each matmul accumulates within a bank.
            ps1 = pp.tile([1, ICH], f32, tag="s1", name="ps1")

            if p == 0:
                # Dense burst so the HAM activity monitor lifts the PE clock
                # gate to 8/8 before the steady-state matmul cadence begins.
                for _ in range(WARMUP_MM):
                    nc.tensor.matmul(
                        ps1[0:1, 0:MMN], lhsT=wrm_w, rhs=wrm_x,
                        start=True, stop=True,
                    )

            pending_copy = []
            for jt in range(NJT):
                col = p * NJT + jt
                segments = [(0, ICH, col)]
                for off, w, segcol in segments:
                    T = tp.tile([128, w], f16, tag="T", name="T")
                    nc.scalar.activation(
                        out=T,
                        in_=mbh[:, off : off + w],
                        func=mybir.ActivationFunctionType.Tanh,
                        scale=tm2_sb[:, jt : jt + 1],
                    )
                    # Emit deferred ACT-side reductions AFTER this tanh so a
                    # stalled Copy never blocks tanh issue in ACT's FIFO.
                    while pending_copy and pending_copy[0][0] <= jt - 2:
                        _, W, wcol = pending_copy.pop(0)
                        scrC = sp.tile([128, ICH], f16, tag="scrC", name="scrC")
                        nc.scalar.activation(
                            out=scrC,
                            in_=W,
                            func=mybir.ActivationFunctionType.Copy,
                            accum_out=accS2[:, wcol : wcol + 1],
                        )
                    for k in range(w // MMN):
                        nc.tensor.matmul(
                            ps1[0:1, off + k * MMN : off + (k + 1) * MMN],
                            lhsT=wm2_sb[:, jt : jt + 1],
                            rhs=T[:, k * MMN : (k + 1) * MMN],
                            start=(jt == 0),
                            stop=(jt == NJT - 1),
                        )
                    if col % OFFLOAD_EVERY == 3 and len(segments) == 1:
                        W = sp.tile([128, ICH], f16, tag="scrW", name="scrW")
                        nc.vector.tensor_tensor(
                            out=W, in0=T, in1=wb16, op=mybir.AluOpType.mult
                        )
                        pending_copy.append((jt, W, col))
                    else:
                        scr = sp.tile([128, w], f16, tag="scr", name="scr")
                        nc.vector.scalar_tensor_tensor(
                            out=scr,
                            in0=T,
                            scalar=1.0,
                            in1=wb16[:, off : off + w],
                            op0=mybir.AluOpType.mult,
                            op1=mybir.AluOpType.mult,
                            accum_out=accS2[:, segcol : segcol + 1],
                        )
            for _, W, wcol in pending_copy:
                scrC = sp.tile([128, ICH], f16, tag="scrC", name="scrC")
                nc.scalar.activation(
                    out=scrC,
                    in_=W,
                    func=mybir.ActivationFunctionType.Copy,
                    accum_out=accS2[:, wcol : wcol + 1],
                )

            s1sb = sp.tile([1, ICH], f32, tag="s1sb", name="s1sb")
            if p == 0:
                nc.scalar.copy(out=s1sb[0:1, :], in_=ps1[0:1, :])
            else:
                # DVE is idle at the tail; keep the final evac off ACT
                nc.vector.tensor_copy(out=s1sb[0:1, :], in_=ps1[0:1, :])
            nc.sync.dma_start(
                out=d_s1.ap()[lo : lo + ICH].unsqueeze(0), in_=s1sb[0:1, :]
            )

        nc.sync.dma_start(out=d_s2.ap(), in_=accS2[:, :])

    nc.compile()
    return nc


def _get_program():
    if "nc" not in _CACHE:
        _CACHE["nc"] = _build_program()
    return _CACHE["nc"]


def _make_in_maps(m1, m2, sc):
    in_maps = []
    for c in range(NCORES):
        b, h = divmod(c, 2)
        js = slice(h * JHALF, (h + 1) * JHALF)
        m2s = m2[b, js].astype(np.float64)
        m1s = m1[b].astype(np.float64)
        in_maps.append(
            {
                "m1h": m1[b].astype(np.float16),
                "wm1": (sc["w1"] * m1s + sc["b1"]).astype(np.float16),
                "tm2": (sc["w0"] * m2s + sc["b0"]).astype(np.float32),
                "wm2": (sc["w2"] * m2s + sc["b2"]).astype(np.float16),
            }
        )
    return in_maps


def _run_device(inputs, trace=False):
    from concourse.bass_utils import run_bass_kernel_spmd

    nc = _get_program()
    m1 = np.asarray(inputs["m1_t"], np.float32)[..., 0]  # [B, N]
    m2 = np.asarray(inputs["m2_t"], np.float32)[..., 0]
    sc = {k: float(np.asarray(inputs[k])) for k in _SCALARS}
    in_maps = _make_in_maps(m1, m2, sc)
    res = run_bass_kernel_spmd(nc, in_maps, list(range(NCORES)), trace=trace)
    return res, m1, m2, sc


def _postprocess(results, m1, m2, sc):
    out1 = np.zeros((B, N), np.float32)
    out2 = np.zeros((B, N), np.float32)
    for b in range(B):
        halves = [results[2 * b], results[2 * b + 1]]
        s1 = (
            halves[0]["o_s1"].astype(np.float64)
            + halves[1]["o_s1"].astype(np.float64)
        )

        def s2_of(r):
            S = r["o_s2"].astype(np.float64)
            S[:, 0] += S[:, IPASS * NJT]  # aux col: 2nd half of split tile 0
            S = S[:, :NJT] + S[:, NJT : IPASS * NJT]  # sum passes; j = jt*128+p
            return S.T.reshape(-1)        # [JHALF]

        s2 = np.concatenate([s2_of(halves[0]), s2_of(halves[1])])

        m1b = m1[b].astype(np.float64)
        m2b = m2[b].astype(np.float64)
        a_m1 = np.tanh(sc["w1"] * m1b + sc["b1"] + s1)
        a_m2 = np.tanh(sc["w2"] * m2b + sc["b2"] + s2)
        l1 = a_m1 * sc["w3"] + sc["b3"]
        l2 = a_m2 * sc["w4"] + sc["b4"]
        e1 = np.exp(l1 - l1.max())
        e2 = np.exp(l2 - l2.max())
        out1[b] = (e1 / e1.sum() * m1b).astype(np.float32)
        out2[b] = (e2 / e2.sum() * m2b).astype(np.float32)
    return out1, out2


def kernel(**inputs):
    res, m1, m2, sc = _run_device(inputs, trace=False)
    return _postprocess(res.results, m1, m2, sc)
